# revision 51
# baseline (speedup 1.0000x reference)
"""Multi-head attention (B=4, S=2048, D=1024, H=16, d_k=64) on 8 TRN2 NeuronCores.

Sharding: batch x head-group. Core c handles batch b = c//2 and heads
[8*(c%2), 8*(c%2)+8). Each core computes Q/K/V projections for its 512
output features (column-parallel), attention for its 8 heads, and a
row-parallel partial of the W_o output projection. The host sums the two
partials per batch (the row-parallel unshard) — no collectives needed.

Device schedule (per core): one software-pipelined stream. Attention is
split into 256 groups (4 q-chunks x 4 head-pairs x 16 key-blocks). Per
group the PE runs a row-tiled score pair (both heads share one N=512
moving pass) and 2 attn@V matmuls; the ACT engine runs one 1024-free
exp per group (~274us total). Scores for TWO consecutive groups are
emitted back-to-back so the ~120ns post-row-tiled-pair drain stall is
paid once per 2 groups. All other PE work (Q/K/V projections, W_o,
normalization broadcasts) is a deadline-sorted task queue pumped ~2
matmuls per group through a 1-bank misc slot, keeping the PE gapless
(p-state: any PE gap drops the clock 2.4->1.2GHz for ~3us, so dummy
matmuls on a memset tile hold the clock hot through the DMA-bound
prologue and the final reciprocal chain). All DMAs ride the single SP
trigger ring in strict first-consumption order: the ring drains FIFO
at the aggregate ~360GB/s, so order IS the bandwidth schedule for the
11.5MB the first 16 attention groups touch; late tensors (xq1-3, wo)
sit at the ring tail. Output partials are written bf16 (host sums the
two head-group partials in fp32), halving the tail flush. Softmax
denominators ride as a 65th V
row; per pair they are DMA-repacked [1,1024]->[8,128] so the DVE
reciprocal_approx_fast (18-bit, plenty for bf16 downstream) takes
~0.3us, then broadcast across partitions by four [8,128]-stationary
mask matmuls; normalization and W_o are deferred (~1.5 pairs / 1
q-chunk) so their inputs are always long ready. PSUM: 2 rotating
2-bank score slots + 2 attn@V accumulator banks + 2 misc banks.
"""

import os
from collections import defaultdict

import numpy as np
import ml_dtypes

import concourse.bacc as bacc
import concourse.mybir as mybir
import concourse.tile as tile
from concourse.bass_utils import run_bass_kernel_spmd

BF16 = mybir.dt.bfloat16
F32 = mybir.dt.float32
EXP = mybir.ActivationFunctionType.Exp

B, S, D = 4, 2048, 1024
H, DK = 16, 64
HPC = 8           # heads per core
FPC = HPC * DK    # 512 features per core
NP = 4            # head pairs per core
NB = 8            # din blocks of 128
NKB = 16          # key blocks of 128
QC = 512          # query chunk
NQC = S // QC     # 4
NG = NKB          # groups (1 key block each) per (qc, m)

_nc_cache = None
last_results = None


def build():
    nc = bacc.Bacc("TRN2", target_bir_lowering=False, debug=False, num_devices=8)

    xq = nc.dram_tensor("xq", [D, S], BF16, kind="ExternalInput").ap()
    xk = nc.dram_tensor("xk", [D, S], BF16, kind="ExternalInput").ap()
    xv = nc.dram_tensor("xv", [D, S], BF16, kind="ExternalInput").ap()
    wq = nc.dram_tensor("wq", [D, FPC], BF16, kind="ExternalInput").ap()
    wk = nc.dram_tensor("wk", [D, FPC], BF16, kind="ExternalInput").ap()
    wv = nc.dram_tensor("wv", [D, FPC], BF16, kind="ExternalInput").ap()
    wo = nc.dram_tensor("wo", [FPC, D], BF16, kind="ExternalInput").ap()
    mask = nc.dram_tensor("mask", [8, 512], BF16, kind="ExternalInput").ap()
    # bf16 partials: host sums the two head-group partials in fp32; the
    # ~0.2% partial quantization is far inside the error budget and halves
    # the output DMA flush at the kernel tail.
    out = nc.dram_tensor("out", [S, D], BF16, kind="ExternalOutput").ap()

    SP = nc.sync

    with tile.TileContext(nc) as tc:
        with (
            tc.tile_pool(name="wp", bufs=1) as wp,
            tc.tile_pool(name="qkv", bufs=1) as qkv,
            tc.tile_pool(name="xp", bufs=1) as xp,
            tc.tile_pool(name="xvp", bufs=4) as xvp,
            tc.tile_pool(name="ptp", bufs=4) as ptp,
            tc.tile_pool(name="avsb", bufs=2) as avsb,
            tc.tile_pool(name="otp", bufs=2) as otp,
            tc.tile_pool(name="denp", bufs=2) as denp,
            tc.tile_pool(name="recp", bufs=2) as recp,
            tc.tile_pool(name="outp", bufs=3) as outp,
            tc.tile_pool(name="sp", bufs=2, space="PSUM") as sp,
            tc.tile_pool(name="avp", bufs=2, space="PSUM") as avp,
            tc.tile_pool(name="miscp", bufs=2, space="PSUM") as miscp,
        ):
            # ---- static SBUF tensors ----
            wq_sb = wp.tile([128, NB, NP, 128], BF16, tag="wq")
            wk_sb = wp.tile([128, NB, NP, 128], BF16, tag="wk")
            wv_sb = wp.tile([128, NB, FPC], BF16, tag="wv")
            wo_sb = wp.tile([128, NP, D], BF16, tag="wo")
            m_sb = wp.tile([8, NP, 128], BF16, tag="mask")
            dum_sb = wp.tile([128, 512], BF16, tag="dummy")
            qt_sb = qkv.tile([128, NP, S], BF16, tag="qt")
            kt_sb = qkv.tile([128, NP, S], BF16, tag="kt")
            v_sb = qkv.tile([128, NKB, HPC, 65], BF16, tag="v")
            xq_sb = xp.tile([128, NB, S], BF16, tag="xq")
            xk_sb = xp.tile([128, NB, S], BF16, tag="xk")

            xv_ch = {}
            for c in range(4):
                xv_ch[c] = xvp.tile([128, NB, 512], BF16, tag="xv", name=f"xv{c}")

            # dummy tile + ACT exp-table priming (both engine-local, no DMA)
            nc.vector.memset(dum_sb[:], 0.01)
            nc.vector.memset(v_sb[:, :, :, 64], 1.0)
            warm_act = denp.tile([8, 128], F32, tag="den", name="warmact")
            nc.scalar.activation(warm_act[:, 0:64], dum_sb[0:8, 0:64], EXP,
                                 scale=0.125)

            def dummy_mm(n=512):
                ps = miscp.tile([128, 512], F32, tag="misc", name="warm")
                nc.tensor.matmul(ps[:, 0:n], dum_sb[:, 0:128], dum_sb[:, 0:n],
                                 start=True, stop=True)

            # ---- input DMA units, spread across 3 trigger queues ----
            # Each unit is one dma_start; per-queue emission order is
            # first-consumption order. Units sized so each lands fast on its
            # ~22GB/s DMA queue. (SP ~565ns, ACT ~667ns, GPSIMD ~1us per
            # trigger, serially per queue, running in parallel.)
            def u_w(e, w_sb_, w_, m, b0, nb):
                e.dma_start(
                    w_sb_[:, b0:b0 + nb, m],
                    w_[:, m * 128:(m + 1) * 128]
                    .rearrange("(b p) c -> p b c", p=128)[:, b0:b0 + nb])

            def u_x(e, x_sb_, x_, c, b0, nb, t0=0, nt=512):
                x_sb_dst = x_sb_[:, b0:b0 + nb, c * 512 + t0:c * 512 + t0 + nt]
                e.dma_start(
                    x_sb_dst,
                    x_[:, c * 512 + t0:c * 512 + t0 + nt]
                    .rearrange("(b p) t -> p b t", p=128)[:, b0:b0 + nb])

            def u_wv(e, b0, nb):
                e.dma_start(wv_sb[:, b0:b0 + nb],
                            wv.rearrange("(b p) c -> p b c", p=128)[:, b0:b0 + nb])

            def u_xv(e, c, t0, nt):
                e.dma_start(
                    xv_ch[c][:, :, t0:t0 + nt],
                    xv[:, c * 512 + t0:c * 512 + t0 + nt]
                    .rearrange("(b p) t -> p b t", p=128))

            def u_wo(e, f0, nf):
                e.dma_start(wo_sb[:, f0:f0 + nf],
                            wo.rearrange("(f p) c -> p f c", p=128)[:, f0:f0 + nf])

            # DMA plan (single SP ring, baseline-proven): the ring drains
            # dma_starts in FIFO order at aggregate ~360GB/s, and the ring
            # has bounded capacity — the sequencer backpressures when it is
            # full, so trigger issue self-paces to drain rate. A single
            # ring in exact first-consumption order is therefore optimal
            # for the bandwidth-bound first ~30us (the first 16 attention
            # groups touch all of K and V: ~11.5MB). Splitting across the
            # ACT ring halves SP's drain share and delays the stream;
            # GPSIMD (software DGE) DMA is NOT dependency-tracked by the
            # tile framework — never use either for loads.
            SP.dma_start(m_sb[:], mask.rearrange("p (j c) -> p j c", c=128))
            u_w(SP, wq_sb, wq, 0, 0, 4)
            u_w(SP, wq_sb, wq, 0, 4, 4)
            for b in range(4):
                u_x(SP, xq_sb, xq, 0, b, 1)
            u_w(SP, wk_sb, wk, 0, 0, 4)
            u_w(SP, wk_sb, wk, 0, 4, 4)
            for b in range(4, 8):
                u_x(SP, xq_sb, xq, 0, b, 1)
            for b in range(6):
                u_x(SP, xk_sb, xk, 0, b, 1)
            u_wv(SP, 0, 4)
            u_wv(SP, 4, 4)
            u_x(SP, xk_sb, xk, 0, 6, 2)
            for t in range(4):
                u_xv(SP, 0, 128 * t, 128)
            u_x(SP, xk_sb, xk, 1, 0, 4)
            u_x(SP, xk_sb, xk, 1, 4, 4)
            for t in range(4):
                u_xv(SP, 1, 128 * t, 128)
            u_x(SP, xk_sb, xk, 2, 0, 4)
            u_x(SP, xk_sb, xk, 2, 4, 4)
            u_x(SP, xk_sb, xk, 3, 0, 4)
            u_x(SP, xk_sb, xk, 3, 4, 4)
            u_w(SP, wq_sb, wq, 1, 0, NB)
            u_w(SP, wk_sb, wk, 1, 0, NB)
            for t in range(4):
                u_xv(SP, 2, 128 * t, 128)
            for t in range(4):
                u_xv(SP, 3, 128 * t, 128)
            u_w(SP, wq_sb, wq, 2, 0, NB)
            u_w(SP, wk_sb, wk, 2, 0, NB)
            u_w(SP, wq_sb, wq, 3, 0, NB)
            u_w(SP, wk_sb, wk, 3, 0, NB)
            u_x(SP, xq_sb, xq, 1, 0, 4)
            u_x(SP, xq_sb, xq, 1, 4, 4)
            u_wo(SP, 0, 2)
            u_wo(SP, 2, 2)
            u_x(SP, xq_sb, xq, 2, 0, 4)
            u_x(SP, xq_sb, xq, 2, 4, 4)
            u_x(SP, xq_sb, xq, 3, 0, 4)
            u_x(SP, xq_sb, xq, 3, 4, 4)

            # ---- filler tasks: generators yielding one PE matmul per step ----
            def proj_chunk_task(x_sb, w_sb, dst, m, c, act_evac=False):
                """512-token projection chunk: 8 matmuls + evac copy.

                Early tasks (first-q-chunk window, where the PE is the
                bottleneck and ACT has ~20% slack) evacuate via the ACT
                engine: a congested DVE queue otherwise delays the misc
                PSUM slot hand-off and inflates the next task's first
                matmul by ~120ns."""
                ps = miscp.tile([128, 512], F32, tag="misc", name="projps")
                lo = c * 512
                for b in range(NB):
                    nc.tensor.matmul(ps[:], w_sb[:, b, m], x_sb[:, b, lo:lo + 512],
                                     start=(b == 0), stop=(b == NB - 1))
                    if b < NB - 1:
                        yield
                eng = nc.scalar if act_evac else nc.vector
                eng_copy = eng.copy if act_evac else eng.tensor_copy
                eng_copy(dst[:, m, lo:lo + 512], ps[:])

            def v_chunk_task(tt):
                """V projection for token tile tt: 8 matmuls + evac copy."""
                ps = miscp.tile([128, 512], F32, tag="misc", name="vps")
                ch = xv_ch[tt // 4]
                off = (tt % 4) * 128
                for b in range(NB):
                    nc.tensor.matmul(ps[:], ch[:, b, off:off + 128], wv_sb[:, b],
                                     start=(b == 0), stop=(b == NB - 1))
                    if b < NB - 1:
                        yield
                nc.scalar.copy(
                    v_sb[:, tt, :, 0:64],
                    ps[:].rearrange("p (h c) -> p h c", c=64))

            ot_tiles = {}
            avsb_tiles = {}
            rec_tiles = {}

            def norm_task(qc, m):
                """Normalize pair (qc, m): broadcast 1/den via mask matmul, mul."""
                rec2 = rec_tiles[(qc, m)]
                if m == 0:
                    ot_tiles[qc] = otp.tile([128, NP, QC], BF16, tag="ot", name="ot")
                ot = ot_tiles[qc]
                av_sb = avsb_tiles.pop((qc, m))
                scp = miscp.tile([128, 512], F32, tag="misc", name="scp")
                for j in range(4):
                    nc.tensor.matmul(scp[:, 128 * j:128 * (j + 1)], m_sb[:, j],
                                     rec2[:], start=True, stop=True)
                nc.vector.tensor_mul(ot[0:64, m], av_sb[0:64, 0:QC], scp[0:64, 0:QC])
                nc.vector.tensor_mul(ot[64:128, m], av_sb[0:64, QC:2 * QC], scp[64:128, 0:QC])
                return
                yield

            def wo_task(qc, tt, jc):
                """Half of the output projection for token block (qc, tt)."""
                ot = ot_tiles[qc]
                wop = miscp.tile([128, 512], F32, tag="misc", name="wop")
                tsl = slice(tt * 128, (tt + 1) * 128)
                for fb in range(NP):
                    nc.tensor.matmul(
                        wop[:], ot[:, fb, tsl], wo_sb[:, fb, jc * 512:(jc + 1) * 512],
                        start=(fb == 0), stop=(fb == NP - 1))
                    if fb < NP - 1:
                        yield
                # per-(tt,jc) staging halves: outp bufs=3 rotation means a
                # copy only waits for the DMA three halves back. At the
                # drain the evacs alternate ACT/DVE so neither serializes.
                ostage = outp.tile([128, 512], BF16, tag="ostage", name="ostage")
                if qc == NQC - 1 and (tt + jc) % 2:
                    nc.scalar.copy(ostage[:], wop[:])
                else:
                    nc.vector.tensor_copy(ostage[:], wop[:])
                row = qc * QC + tt * 128
                SP.dma_start(out[row:row + 128, jc * 512:(jc + 1) * 512],
                             ostage[:])

            ostage_tiles = {}

            # Task queue: (due_index, generator). Pump keeps deadline order.
            tasks = []
            def add_task(due, gen):
                tasks.append((due, gen))
                tasks.sort(key=lambda t: t[0])

            open_task = None
            open_due = 0

            def pump(i, min_steps):
                """Emit filler matmul-steps: min_steps per index, running
                ahead only to finish overdue tasks (deadline-driven)."""
                nonlocal open_task, open_due
                emitted = 0
                while True:
                    if open_task is None and tasks and (
                            emitted < min_steps or tasks[0][0] <= i + 2):
                        open_due, open_task = tasks.pop(0)
                    if open_task is None:
                        return
                    if emitted >= min_steps and open_due > i:
                        return
                    try:
                        next(open_task)
                    except StopIteration:
                        open_task = None
                    emitted += 1

            # Due = first index whose scores/av READ the produced tile, minus
            # 5: the pump runs after score emission within each 2-group
            # macro, so tasks must finish a full macro earlier than with the
            # old per-group loop to keep consumers off the evac-copy wait.
            for tt in range(NKB):
                if tt >= 1:
                    add_task(1 + tt, v_chunk_task(tt))
            for m in range(NP):
                for c in range(4):
                    if not (m == 0 and c == 0):
                        add_task(16 * m + 4 * c - 5,
                                 proj_chunk_task(xk_sb, wk_sb, kt_sb, m, c,
                                                 act_evac=True))
            for m in range(NP):
                for c in range(4):
                    if not (m == 0 and c == 0):
                        due = 64 * c + 16 * m - 5
                        add_task(due,
                                 proj_chunk_task(xq_sb, wq_sb, qt_sb, m, c,
                                                 act_evac=(due < 96)))

            fill_at = defaultdict(list)
            # normalization / Wo release points: pair p=(qc, m) spans indices
            # [16p, 16p+16). norm of pair p-2 at +10 (reciprocal chain of p-2
            # finished ~5 indices earlier); Wo of q-chunk qc-1 in (qc, m2/m3).
            for qc in range(NQC):
                for m in range(NP):
                    base = 16 * (4 * qc + m)
                    if m >= 2 or qc > 0:
                        nqc, nm = (qc, m - 2) if m >= 2 else (qc - 1, m + 2)
                        if nqc < NQC - 1:
                            fill_at[base + 10].append(
                                lambda nqc=nqc, nm=nm: norm_task(nqc, nm))
                    if qc > 0 and m >= 2:
                        tt0 = 2 * (m - 2)
                        fill_at[base + 2].append(
                            lambda q=qc - 1, tt=tt0: wo_task(q, tt, 0))
                        fill_at[base + 5].append(
                            lambda q=qc - 1, tt=tt0: wo_task(q, tt, 1))
                        fill_at[base + 9].append(
                            lambda q=qc - 1, tt=tt0 + 1: wo_task(q, tt, 0))
                        fill_at[base + 12].append(
                            lambda q=qc - 1, tt=tt0 + 1: wo_task(q, tt, 1))
            # last q-chunk: per-pair reciprocal lets each norm release ~6
            # indices after its pair ends (norm(3,3) + all Wo(3,*) drain)
            for m in range(NP - 1):
                fill_at[16 * (4 * (NQC - 1) + m) + 22].append(
                    lambda nm=m: norm_task(NQC - 1, nm))


            # ---- prologue: warm the PE p-state on dummy matmuls while the
            # first DMA chunks land, then minimum projections for attention.
            # Dummies are interleaved between the DMA-paced first matmuls so
            # the clock never drops (a gap halves the PE rate for ~3us).
            for _ in range(16):
                dummy_mm(128)
            for _ in range(8):
                dummy_mm(512)

            def prologue_chunk(gen, fillers):
                for _ in gen:
                    for _ in range(fillers):
                        dummy_mm(512)

            prologue_chunk(proj_chunk_task(xq_sb, wq_sb, qt_sb, 0, 0), 2)
            for _ in range(3):
                dummy_mm(512)
            prologue_chunk(proj_chunk_task(xk_sb, wk_sb, kt_sb, 0, 0), 1)
            # v(0) is not needed until av(0) in macro 2 — pump it there so
            # scores(0)/exp(0) start as soon as q00/k00 are projected
            add_task(-1, v_chunk_task(0))

            # ---- main attention pipeline ----
            groups = [(qc, m, g) for qc in range(NQC) for m in range(NP)
                      for g in range(NG)]
            NGRP = len(groups)
            SKEW = 3
            pt_tiles = {}
            av_tiles = {}

            def emit_scores(gi):
                qc, m, g = groups[gi]
                qsl = slice(qc * QC, (qc + 1) * QC)
                ksl = slice(g * 128, (g + 1) * 128)
                s = sp.tile([128, 1024], F32, tag="s", name="s")
                nc.tensor.matmul(s[:, 0:512], kt_sb[0:64, m, ksl], qt_sb[0:64, m, qsl],
                                 start=True, stop=True, tile_position=(0, 0))
                nc.tensor.matmul(s[:, 512:1024], kt_sb[64:128, m, ksl], qt_sb[64:128, m, qsl],
                                 start=True, stop=True, tile_position=(64, 0))
                pt = ptp.tile([128, 1024], BF16, tag="pt", name="pt")
                pt_tiles[gi] = pt
                nc.scalar.activation(pt[:], s[:], EXP, scale=0.125)

            def emit_av(gi):
                qc, m, g = groups[gi]
                pt = pt_tiles.pop(gi)
                if g == 0:
                    av_tiles[(qc, m, 0)] = avp.tile([128, QC], F32, tag="av", name="avA")
                    av_tiles[(qc, m, 1)] = avp.tile([128, QC], F32, tag="av", name="avB")
                avA = av_tiles[(qc, m, 0)]
                avB = av_tiles[(qc, m, 1)]
                nc.tensor.matmul(avA[0:65, :], v_sb[:, g, 2 * m, 0:65], pt[:, 0:512],
                                 start=(g == 0), stop=(g == NG - 1))
                nc.tensor.matmul(avB[0:65, :], v_sb[:, g, 2 * m + 1, 0:65], pt[:, 512:1024],
                                 start=(g == 0), stop=(g == NG - 1))
                if g == NG - 1:
                    pair_end(qc, m)

            def pair_end(qc, m):
                last = (qc == NQC - 1 and m == NP - 1)
                avA = av_tiles.pop((qc, m, 0))
                avB = av_tiles.pop((qc, m, 1))
                av_sb = avsb.tile([128, 2 * QC], F32, tag="av_sb", name="av_sb")
                avsb_tiles[(qc, m)] = av_sb
                den = denp.tile([8, 128], F32, tag="den", name="den")
                # denominators: [1, 512] -> [4, 128] repack per head keeps the
                # reciprocal's free size small; head A lands while head B's
                # copy still runs on the DVE.
                nc.vector.tensor_copy(av_sb[0:65, 0:QC], avA[0:65, :])
                SP.dma_start(den[0:4], av_sb[64:65, 0:QC])
                nc.vector.tensor_copy(av_sb[0:65, QC:2 * QC], avB[0:65, :])
                SP.dma_start(den[4:8], av_sb[64:65, QC:2 * QC])
                recf = recp.tile([8, 128], F32, tag="recf", name="recf")
                nc.vector.reciprocal_approx_fast(recf[:], den[:])
                rec2 = recp.tile([8, 128], BF16, tag="rec2", name="rec2")
                nc.vector.tensor_copy(rec2[:], recf[:])
                rec_tiles[(qc, m)] = rec2

            av_cursor = 0
            for j in range(0, NGRP + 4, 2):
                for i in (j, j + 1):
                    for f in fill_at.pop(i, []):
                        add_task(i, f())
                if j < NGRP:
                    emit_scores(j)
                    emit_scores(j + 1)
                if j < 24:
                    if j < 12:
                        dummy_mm(512)   # p-state insurance in early DMA waits
                    pump(j + 1, 6)
                    while av_cursor <= min(j + 1 - SKEW, NGRP - 1):
                        emit_av(av_cursor)
                        av_cursor += 1
                else:
                    quota = 4 if j < NGRP else NGRP
                    while quota and av_cursor <= min(j + 1 - SKEW, NGRP - 1):
                        emit_av(av_cursor)
                        av_cursor += 1
                        quota -= 1
                    pump(j + 1, 4)

            # ---- drain: remaining tasks, dummies to keep the PE clock hot
            # through the last pair's reciprocal chain, then norm + Wo ----
            pump(10 ** 9, 10 ** 9)
            for _ in range(16):
                dummy_mm(512)
            for gen in ([norm_task(NQC - 1, 3)] +
                        [wo_task(NQC - 1, tt, jc) for tt in range(4) for jc in range(2)]):
                for _ in gen:
                    pass

    nc.compile()
    return nc


def _get_nc():
    global _nc_cache
    if _nc_cache is None:
        _nc_cache = build()
    return _nc_cache


def kernel(query, key, value, W_q, W_k, W_v, W_o):
    global last_results
    nc = _get_nc()
    bf = ml_dtypes.bfloat16

    # broadcast masks for the [8, 128]-packed reciprocals: output column
    # block j picks row j (head A, partitions 0:64) / row 4+j (head B)
    mask = np.zeros((8, 512), bf)
    for j in range(4):
        mask[j, j * 128:j * 128 + 64] = 1.0
        mask[4 + j, j * 128 + 64:j * 128 + 128] = 1.0

    in_maps = []
    xt = {}
    for b in range(B):
        xt[b] = {
            "xq": np.ascontiguousarray(query[b].T).astype(bf),
            "xk": np.ascontiguousarray(key[b].T).astype(bf),
            "xv": np.ascontiguousarray(value[b].T).astype(bf),
        }
    wmaps = []
    for hg in range(2):
        r = slice(hg * FPC, (hg + 1) * FPC)
        wmaps.append({
            "wq": np.ascontiguousarray(W_q[r, :].T).astype(bf),
            "wk": np.ascontiguousarray(W_k[r, :].T).astype(bf),
            "wv": np.ascontiguousarray(W_v[r, :].T).astype(bf),
            "wo": np.ascontiguousarray(W_o[:, r].T).astype(bf),
        })
    for c in range(8):
        b, hg = c // 2, c % 2
        in_maps.append({**xt[b], **wmaps[hg], "mask": mask})

    res = run_bass_kernel_spmd(
        nc, in_maps, core_ids=list(range(8)),
        trace=bool(os.environ.get("BASS_KERNEL_TRACE")))
    last_results = res

    out = np.empty((B, S, D), np.float32)
    for b in range(B):
        out[b] = (res.results[2 * b]["out"].astype(np.float32)
                  + res.results[2 * b + 1]["out"].astype(np.float32))
    return out


# revision 55
# speedup vs baseline: 1.0092x; 1.0092x over previous
"""Multi-head attention (B=4, S=2048, D=1024, H=16, d_k=64) on 8 TRN2 NeuronCores.

Sharding: batch x head-group. Core c handles batch b = c//2 and heads
[8*(c%2), 8*(c%2)+8). Each core computes Q/K/V projections for its 512
output features (column-parallel), attention for its 8 heads, and a
row-parallel partial of the W_o output projection. The host sums the two
partials per batch (the row-parallel unshard) — no collectives needed.

Device schedule (per core): one software-pipelined stream. Attention is
split into 256 groups (4 q-chunks x 4 head-pairs x 16 key-blocks). Per
group the PE runs a row-tiled score pair (both heads share one N=512
moving pass) and 2 attn@V matmuls; the ACT engine runs one 1024-free
exp per group (~274us total). Scores for TWO consecutive groups are
emitted back-to-back so the ~120ns post-row-tiled-pair drain stall is
paid once per 2 groups. All other PE work (Q/K/V projections, W_o,
normalization broadcasts) is a deadline-sorted task queue pumped ~2
matmuls per group through a 1-bank misc slot, keeping the PE gapless
(p-state: any PE gap drops the clock 2.4->1.2GHz for ~3us, so dummy
matmuls on a memset tile hold the clock hot through the DMA-bound
prologue and the final reciprocal chain). All DMAs ride the single SP
trigger ring in strict first-consumption order: the ring drains FIFO
at the aggregate ~360GB/s, so order IS the bandwidth schedule for the
11.5MB the first 16 attention groups touch; late tensors (xq1-3, wo)
sit at the ring tail. Output partials are written bf16 (host sums the
two head-group partials in fp32), halving the tail flush. Softmax
denominators ride as a 65th V
row; per pair they are DMA-repacked [1,1024]->[8,128] so the DVE
reciprocal_approx_fast (18-bit, plenty for bf16 downstream) takes
~0.3us, then broadcast across partitions by four [8,128]-stationary
mask matmuls; normalization and W_o are deferred (~1.5 pairs / 1
q-chunk) so their inputs are always long ready. PSUM: 2 rotating
2-bank score slots + 2 attn@V accumulator banks + 2 misc banks.
"""

import os
from collections import defaultdict

import numpy as np
import ml_dtypes

import concourse.bacc as bacc
import concourse.mybir as mybir
import concourse.tile as tile
from concourse.bass_utils import run_bass_kernel_spmd

BF16 = mybir.dt.bfloat16
F32 = mybir.dt.float32
EXP = mybir.ActivationFunctionType.Exp

B, S, D = 4, 2048, 1024
H, DK = 16, 64
HPC = 8           # heads per core
FPC = HPC * DK    # 512 features per core
NP = 4            # head pairs per core
NB = 8            # din blocks of 128
NKB = 16          # key blocks of 128
QC = 512          # query chunk
NQC = S // QC     # 4
NG = NKB          # groups (1 key block each) per (qc, m)

_nc_cache = None
last_results = None


def build():
    nc = bacc.Bacc("TRN2", target_bir_lowering=False, debug=False, num_devices=8)

    xq = nc.dram_tensor("xq", [D, S], BF16, kind="ExternalInput").ap()
    xk = nc.dram_tensor("xk", [D, S], BF16, kind="ExternalInput").ap()
    xv = nc.dram_tensor("xv", [D, S], BF16, kind="ExternalInput").ap()
    wq = nc.dram_tensor("wq", [D, FPC], BF16, kind="ExternalInput").ap()
    wk = nc.dram_tensor("wk", [D, FPC], BF16, kind="ExternalInput").ap()
    wv = nc.dram_tensor("wv", [D, FPC], BF16, kind="ExternalInput").ap()
    wo = nc.dram_tensor("wo", [FPC, D], BF16, kind="ExternalInput").ap()
    mask = nc.dram_tensor("mask", [8, 512], BF16, kind="ExternalInput").ap()
    # bf16 partials: host sums the two head-group partials in fp32; the
    # ~0.2% partial quantization is far inside the error budget and halves
    # the output DMA flush at the kernel tail.
    out = nc.dram_tensor("out", [S, D], BF16, kind="ExternalOutput").ap()

    SP = nc.sync

    with tile.TileContext(nc) as tc:
        with (
            tc.tile_pool(name="wp", bufs=1) as wp,
            tc.tile_pool(name="qkv", bufs=1) as qkv,
            tc.tile_pool(name="xp", bufs=1) as xp,
            tc.tile_pool(name="xvp", bufs=4) as xvp,
            tc.tile_pool(name="ptp", bufs=4) as ptp,
            tc.tile_pool(name="avsb", bufs=2) as avsb,
            tc.tile_pool(name="otp", bufs=2) as otp,
            tc.tile_pool(name="denp", bufs=2) as denp,
            tc.tile_pool(name="recp", bufs=2) as recp,
            tc.tile_pool(name="outp", bufs=3) as outp,
            tc.tile_pool(name="sp", bufs=2, space="PSUM") as sp,
            tc.tile_pool(name="avp", bufs=2, space="PSUM") as avp,
            tc.tile_pool(name="miscp", bufs=2, space="PSUM") as miscp,
        ):
            # ---- static SBUF tensors ----
            wq_sb = wp.tile([128, NB, NP, 128], BF16, tag="wq")
            wk_sb = wp.tile([128, NB, NP, 128], BF16, tag="wk")
            wv_sb = wp.tile([128, NB, FPC], BF16, tag="wv")
            wo_sb = wp.tile([128, NP, D], BF16, tag="wo")
            m_sb = wp.tile([8, NP, 128], BF16, tag="mask")
            dum_sb = wp.tile([128, 512], BF16, tag="dummy")
            qt_sb = qkv.tile([128, NP, S], BF16, tag="qt")
            kt_sb = qkv.tile([128, NP, S], BF16, tag="kt")
            v_sb = qkv.tile([128, NKB, HPC, 65], BF16, tag="v")
            xq_sb = xp.tile([128, NB, S], BF16, tag="xq")
            xk_sb = xp.tile([128, NB, S], BF16, tag="xk")

            xv_ch = {}
            for c in range(4):
                xv_ch[c] = xvp.tile([128, NB, 512], BF16, tag="xv", name=f"xv{c}")

            # dummy tile + ACT exp-table priming (both engine-local, no DMA)
            nc.vector.memset(dum_sb[:], 0.01)
            nc.vector.memset(v_sb[:, :, :, 64], 1.0)
            warm_act = denp.tile([8, 128], F32, tag="den", name="warmact")
            nc.scalar.activation(warm_act[:, 0:64], dum_sb[0:8, 0:64], EXP,
                                 scale=0.125)

            def dummy_mm(n=512):
                ps = miscp.tile([128, 512], F32, tag="misc", name="warm")
                nc.tensor.matmul(ps[:, 0:n], dum_sb[:, 0:128], dum_sb[:, 0:n],
                                 start=True, stop=True)

            # ---- input DMA units, spread across 3 trigger queues ----
            # Each unit is one dma_start; per-queue emission order is
            # first-consumption order. Units sized so each lands fast on its
            # ~22GB/s DMA queue. (SP ~565ns, ACT ~667ns, GPSIMD ~1us per
            # trigger, serially per queue, running in parallel.)
            def u_w(e, w_sb_, w_, m, b0, nb):
                e.dma_start(
                    w_sb_[:, b0:b0 + nb, m],
                    w_[:, m * 128:(m + 1) * 128]
                    .rearrange("(b p) c -> p b c", p=128)[:, b0:b0 + nb])

            def u_x(e, x_sb_, x_, c, b0, nb, t0=0, nt=512):
                x_sb_dst = x_sb_[:, b0:b0 + nb, c * 512 + t0:c * 512 + t0 + nt]
                e.dma_start(
                    x_sb_dst,
                    x_[:, c * 512 + t0:c * 512 + t0 + nt]
                    .rearrange("(b p) t -> p b t", p=128)[:, b0:b0 + nb])

            def u_wv(e, b0, nb):
                e.dma_start(wv_sb[:, b0:b0 + nb],
                            wv.rearrange("(b p) c -> p b c", p=128)[:, b0:b0 + nb])

            def u_xv(e, c, t0, nt):
                e.dma_start(
                    xv_ch[c][:, :, t0:t0 + nt],
                    xv[:, c * 512 + t0:c * 512 + t0 + nt]
                    .rearrange("(b p) t -> p b t", p=128))

            def u_wo(e, f0, nf):
                e.dma_start(wo_sb[:, f0:f0 + nf],
                            wo.rearrange("(f p) c -> p f c", p=128)[:, f0:f0 + nf])

            # DMA plan (single SP ring, baseline-proven): the ring drains
            # dma_starts in FIFO order at aggregate ~360GB/s, and the ring
            # has bounded capacity — the sequencer backpressures when it is
            # full, so trigger issue self-paces to drain rate. A single
            # ring in exact first-consumption order is therefore optimal
            # for the bandwidth-bound first ~30us (the first 16 attention
            # groups touch all of K and V: ~11.5MB). Splitting across the
            # ACT ring halves SP's drain share and delays the stream;
            # GPSIMD (software DGE) DMA is NOT dependency-tracked by the
            # tile framework — never use either for loads.
            SP.dma_start(m_sb[:], mask.rearrange("p (j c) -> p j c", c=128))
            u_w(SP, wq_sb, wq, 0, 0, 4)
            u_w(SP, wq_sb, wq, 0, 4, 4)
            for b in range(4):
                u_x(SP, xq_sb, xq, 0, b, 1)
            u_w(SP, wk_sb, wk, 0, 0, 4)
            u_w(SP, wk_sb, wk, 0, 4, 4)
            for b in range(4, 8):
                u_x(SP, xq_sb, xq, 0, b, 1)
            for b in range(6):
                u_x(SP, xk_sb, xk, 0, b, 1)
            u_wv(SP, 0, 4)
            u_wv(SP, 4, 4)
            u_x(SP, xk_sb, xk, 0, 6, 2)
            for t in range(4):
                u_xv(SP, 0, 128 * t, 128)
            u_x(SP, xk_sb, xk, 1, 0, 4)
            u_x(SP, xk_sb, xk, 1, 4, 4)
            for t in range(4):
                u_xv(SP, 1, 128 * t, 128)
            u_x(SP, xk_sb, xk, 2, 0, 4)
            u_x(SP, xk_sb, xk, 2, 4, 4)
            u_x(SP, xk_sb, xk, 3, 0, 4)
            u_x(SP, xk_sb, xk, 3, 4, 4)
            u_w(SP, wq_sb, wq, 1, 0, NB)
            u_w(SP, wk_sb, wk, 1, 0, NB)
            for t in range(4):
                u_xv(SP, 2, 128 * t, 128)
            for t in range(4):
                u_xv(SP, 3, 128 * t, 128)
            u_w(SP, wq_sb, wq, 2, 0, NB)
            u_w(SP, wk_sb, wk, 2, 0, NB)
            u_w(SP, wq_sb, wq, 3, 0, NB)
            u_w(SP, wk_sb, wk, 3, 0, NB)
            u_x(SP, xq_sb, xq, 1, 0, 4)
            u_x(SP, xq_sb, xq, 1, 4, 4)
            u_wo(SP, 0, 2)
            u_wo(SP, 2, 2)
            u_x(SP, xq_sb, xq, 2, 0, 4)
            u_x(SP, xq_sb, xq, 2, 4, 4)
            u_x(SP, xq_sb, xq, 3, 0, 4)
            u_x(SP, xq_sb, xq, 3, 4, 4)

            # ---- filler tasks: generators yielding one PE matmul per step ----
            def proj_chunk_task(x_sb, w_sb, dst, m, c, act_evac=False):
                """512-token projection chunk: 8 matmuls + evac copy.

                Early tasks (first-q-chunk window, where the PE is the
                bottleneck and ACT has ~20% slack) evacuate via the ACT
                engine: a congested DVE queue otherwise delays the misc
                PSUM slot hand-off and inflates the next task's first
                matmul by ~120ns."""
                ps = miscp.tile([128, 512], F32, tag="misc", name="projps")
                lo = c * 512
                for b in range(NB):
                    nc.tensor.matmul(ps[:], w_sb[:, b, m], x_sb[:, b, lo:lo + 512],
                                     start=(b == 0), stop=(b == NB - 1))
                    if b < NB - 1:
                        yield
                if act_evac:
                    nc.scalar.copy(dst[:, m, lo:lo + 512], ps[:])
                else:
                    nc.vector.tensor_copy(dst[:, m, lo:lo + 512], ps[:])

            def v_chunk_task(tt):
                """V projection for token tile tt: 8 matmuls + evac copy."""
                ps = miscp.tile([128, 512], F32, tag="misc", name="vps")
                ch = xv_ch[tt // 4]
                off = (tt % 4) * 128
                for b in range(NB):
                    nc.tensor.matmul(ps[:], ch[:, b, off:off + 128], wv_sb[:, b],
                                     start=(b == 0), stop=(b == NB - 1))
                    if b < NB - 1:
                        yield
                nc.vector.tensor_copy(
                    v_sb[:, tt, :, 0:64],
                    ps[:].rearrange("p (h c) -> p h c", c=64))

            ot_tiles = {}
            avsb_tiles = {}
            rec_tiles = {}

            def norm_task(qc, m):
                """Normalize pair (qc, m): broadcast 1/den via mask matmul, mul."""
                rec2 = rec_tiles[(qc, m)]
                if m == 0:
                    ot_tiles[qc] = otp.tile([128, NP, QC], BF16, tag="ot", name="ot")
                ot = ot_tiles[qc]
                av_sb = avsb_tiles.pop((qc, m))
                scp = miscp.tile([128, 512], F32, tag="misc", name="scp")
                for j in range(4):
                    nc.tensor.matmul(scp[:, 128 * j:128 * (j + 1)], m_sb[:, j],
                                     rec2[:], start=True, stop=True)
                nc.vector.tensor_mul(ot[0:64, m], av_sb[0:64, 0:QC], scp[0:64, 0:QC])
                nc.vector.tensor_mul(ot[64:128, m], av_sb[0:64, QC:2 * QC], scp[64:128, 0:QC])
                return
                yield

            def wo_task(qc, tt, jc):
                """Half of the output projection for token block (qc, tt)."""
                ot = ot_tiles[qc]
                wop = miscp.tile([128, 512], F32, tag="misc", name="wop")
                tsl = slice(tt * 128, (tt + 1) * 128)
                for fb in range(NP):
                    nc.tensor.matmul(
                        wop[:], ot[:, fb, tsl], wo_sb[:, fb, jc * 512:(jc + 1) * 512],
                        start=(fb == 0), stop=(fb == NP - 1))
                    if fb < NP - 1:
                        yield
                # per-(tt,jc) staging halves: outp bufs=3 rotation means a
                # copy only waits for the DMA three halves back. At the
                # drain the evacs alternate ACT/DVE so neither serializes.
                ostage = outp.tile([128, 512], BF16, tag="ostage", name="ostage")
                if qc == NQC - 1 and (tt + jc) % 2:
                    nc.scalar.copy(ostage[:], wop[:])
                else:
                    nc.vector.tensor_copy(ostage[:], wop[:])
                row = qc * QC + tt * 128
                SP.dma_start(out[row:row + 128, jc * 512:(jc + 1) * 512],
                             ostage[:])

            ostage_tiles = {}

            # Task queue: (due_index, generator). Pump keeps deadline order.
            tasks = []
            def add_task(due, gen):
                tasks.append((due, gen))
                tasks.sort(key=lambda t: t[0])

            open_task = None
            open_due = 0

            def pump(i, min_steps):
                """Emit filler matmul-steps: min_steps per index, running
                ahead only to finish overdue tasks (deadline-driven)."""
                nonlocal open_task, open_due
                emitted = 0
                while True:
                    if open_task is None and tasks and (
                            emitted < min_steps or tasks[0][0] <= i + 2):
                        open_due, open_task = tasks.pop(0)
                    if open_task is None:
                        return
                    if emitted >= min_steps and open_due > i:
                        return
                    try:
                        next(open_task)
                    except StopIteration:
                        open_task = None
                    emitted += 1

            # Due = first index whose scores/av READ the produced tile, minus
            # 5: the pump runs after score emission within each 2-group
            # macro, so tasks must finish a full macro earlier than with the
            # old per-group loop to keep consumers off the evac-copy wait.
            for tt in range(NKB):
                if tt >= 1:
                    add_task(1 + tt, v_chunk_task(tt))
            for m in range(NP):
                for c in range(4):
                    if not (m == 0 and c == 0):
                        add_task(16 * m + 4 * c - 5,
                                 proj_chunk_task(xk_sb, wk_sb, kt_sb, m, c))
            for m in range(NP):
                for c in range(4):
                    if not (m == 0 and c == 0):
                        add_task(64 * c + 16 * m - 5,
                                 proj_chunk_task(xq_sb, wq_sb, qt_sb, m, c))

            fill_at = defaultdict(list)
            # normalization / Wo release points: pair p=(qc, m) spans indices
            # [16p, 16p+16). norm of pair p-2 at +10 (reciprocal chain of p-2
            # finished ~5 indices earlier); Wo of q-chunk qc-1 in (qc, m2/m3).
            for qc in range(NQC):
                for m in range(NP):
                    base = 16 * (4 * qc + m)
                    if m >= 2 or qc > 0:
                        nqc, nm = (qc, m - 2) if m >= 2 else (qc - 1, m + 2)
                        if nqc < NQC - 1:
                            fill_at[base + 10].append(
                                lambda nqc=nqc, nm=nm: norm_task(nqc, nm))
                    if qc > 0 and m >= 2:
                        tt0 = 2 * (m - 2)
                        fill_at[base + 2].append(
                            lambda q=qc - 1, tt=tt0: wo_task(q, tt, 0))
                        fill_at[base + 5].append(
                            lambda q=qc - 1, tt=tt0: wo_task(q, tt, 1))
                        fill_at[base + 9].append(
                            lambda q=qc - 1, tt=tt0 + 1: wo_task(q, tt, 0))
                        fill_at[base + 12].append(
                            lambda q=qc - 1, tt=tt0 + 1: wo_task(q, tt, 1))
            # last q-chunk: per-pair reciprocal lets each norm release ~6
            # indices after its pair ends (norm(3,3) + all Wo(3,*) drain)
            for m in range(NP - 1):
                fill_at[16 * (4 * (NQC - 1) + m) + 22].append(
                    lambda nm=m: norm_task(NQC - 1, nm))


            # ---- prologue: warm the PE p-state on dummy matmuls while the
            # first DMA chunks land, then minimum projections for attention.
            # Dummies are interleaved between the DMA-paced first matmuls so
            # the clock never drops (a gap halves the PE rate for ~3us).
            for _ in range(16):
                dummy_mm(128)
            for _ in range(8):
                dummy_mm(512)

            def prologue_chunk(gen, fillers):
                for _ in gen:
                    for _ in range(fillers):
                        dummy_mm(512)

            prologue_chunk(proj_chunk_task(xq_sb, wq_sb, qt_sb, 0, 0), 2)
            for _ in range(3):
                dummy_mm(512)
            prologue_chunk(proj_chunk_task(xk_sb, wk_sb, kt_sb, 0, 0), 1)
            # v(0) is not needed until av(0) in macro 2 — pump it there so
            # scores(0)/exp(0) start as soon as q00/k00 are projected
            add_task(-1, v_chunk_task(0))

            # ---- main attention pipeline ----
            groups = [(qc, m, g) for qc in range(NQC) for m in range(NP)
                      for g in range(NG)]
            NGRP = len(groups)
            SKEW = 3
            pt_tiles = {}
            av_tiles = {}

            def emit_scores(gi):
                qc, m, g = groups[gi]
                qsl = slice(qc * QC, (qc + 1) * QC)
                ksl = slice(g * 128, (g + 1) * 128)
                s = sp.tile([128, 1024], F32, tag="s", name="s")
                nc.tensor.matmul(s[:, 0:512], kt_sb[0:64, m, ksl], qt_sb[0:64, m, qsl],
                                 start=True, stop=True, tile_position=(0, 0))
                nc.tensor.matmul(s[:, 512:1024], kt_sb[64:128, m, ksl], qt_sb[64:128, m, qsl],
                                 start=True, stop=True, tile_position=(64, 0))
                pt = ptp.tile([128, 1024], BF16, tag="pt", name="pt")
                pt_tiles[gi] = pt
                nc.scalar.activation(pt[:], s[:], EXP, scale=0.125)

            def emit_av(gi):
                qc, m, g = groups[gi]
                pt = pt_tiles.pop(gi)
                if g == 0:
                    av_tiles[(qc, m, 0)] = avp.tile([128, QC], F32, tag="av", name="avA")
                    av_tiles[(qc, m, 1)] = avp.tile([128, QC], F32, tag="av", name="avB")
                avA = av_tiles[(qc, m, 0)]
                avB = av_tiles[(qc, m, 1)]
                nc.tensor.matmul(avA[0:65, :], v_sb[:, g, 2 * m, 0:65], pt[:, 0:512],
                                 start=(g == 0), stop=(g == NG - 1))
                nc.tensor.matmul(avB[0:65, :], v_sb[:, g, 2 * m + 1, 0:65], pt[:, 512:1024],
                                 start=(g == 0), stop=(g == NG - 1))
                if g == NG - 1:
                    pair_end(qc, m)

            def pair_end(qc, m):
                last = (qc == NQC - 1 and m == NP - 1)
                avA = av_tiles.pop((qc, m, 0))
                avB = av_tiles.pop((qc, m, 1))
                av_sb = avsb.tile([128, 2 * QC], F32, tag="av_sb", name="av_sb")
                avsb_tiles[(qc, m)] = av_sb
                den = denp.tile([8, 128], F32, tag="den", name="den")
                # denominators: [1, 512] -> [4, 128] repack per head keeps the
                # reciprocal's free size small; head A lands while head B's
                # copy still runs on the DVE.
                nc.vector.tensor_copy(av_sb[0:65, 0:QC], avA[0:65, :])
                SP.dma_start(den[0:4], av_sb[64:65, 0:QC])
                nc.vector.tensor_copy(av_sb[0:65, QC:2 * QC], avB[0:65, :])
                SP.dma_start(den[4:8], av_sb[64:65, QC:2 * QC])
                recf = recp.tile([8, 128], F32, tag="recf", name="recf")
                nc.vector.reciprocal_approx_fast(recf[:], den[:])
                rec2 = recp.tile([8, 128], BF16, tag="rec2", name="rec2")
                nc.vector.tensor_copy(rec2[:], recf[:])
                rec_tiles[(qc, m)] = rec2

            av_cursor = 0
            for j in range(0, NGRP + 4, 2):
                for i in (j, j + 1):
                    for f in fill_at.pop(i, []):
                        add_task(i, f())
                if j < NGRP:
                    emit_scores(j)
                    emit_scores(j + 1)
                if j < 24:
                    if j < 12:
                        dummy_mm(512)   # p-state insurance in early DMA waits
                    pump(j + 1, 6)
                    while av_cursor <= min(j + 1 - SKEW, NGRP - 1):
                        emit_av(av_cursor)
                        av_cursor += 1
                else:
                    quota = 4 if j < NGRP else NGRP
                    while quota and av_cursor <= min(j + 1 - SKEW, NGRP - 1):
                        emit_av(av_cursor)
                        av_cursor += 1
                        quota -= 1
                    pump(j + 1, 4)

            # ---- drain: remaining tasks, dummies to keep the PE clock hot
            # through the last pair's reciprocal chain, then norm + Wo ----
            pump(10 ** 9, 10 ** 9)
            for _ in range(16):
                dummy_mm(512)
            for gen in ([norm_task(NQC - 1, 3)] +
                        [wo_task(NQC - 1, tt, jc) for tt in range(4) for jc in range(2)]):
                for _ in gen:
                    pass

    nc.compile()
    return nc


def _get_nc():
    global _nc_cache
    if _nc_cache is None:
        _nc_cache = build()
    return _nc_cache


def kernel(query, key, value, W_q, W_k, W_v, W_o):
    global last_results
    nc = _get_nc()
    bf = ml_dtypes.bfloat16

    # broadcast masks for the [8, 128]-packed reciprocals: output column
    # block j picks row j (head A, partitions 0:64) / row 4+j (head B)
    mask = np.zeros((8, 512), bf)
    for j in range(4):
        mask[j, j * 128:j * 128 + 64] = 1.0
        mask[4 + j, j * 128 + 64:j * 128 + 128] = 1.0

    in_maps = []
    xt = {}
    for b in range(B):
        xt[b] = {
            "xq": np.ascontiguousarray(query[b].T).astype(bf),
            "xk": np.ascontiguousarray(key[b].T).astype(bf),
            "xv": np.ascontiguousarray(value[b].T).astype(bf),
        }
    wmaps = []
    for hg in range(2):
        r = slice(hg * FPC, (hg + 1) * FPC)
        wmaps.append({
            "wq": np.ascontiguousarray(W_q[r, :].T).astype(bf),
            "wk": np.ascontiguousarray(W_k[r, :].T).astype(bf),
            "wv": np.ascontiguousarray(W_v[r, :].T).astype(bf),
            "wo": np.ascontiguousarray(W_o[:, r].T).astype(bf),
        })
    for c in range(8):
        b, hg = c // 2, c % 2
        in_maps.append({**xt[b], **wmaps[hg], "mask": mask})

    res = run_bass_kernel_spmd(
        nc, in_maps, core_ids=list(range(8)),
        trace=bool(os.environ.get("BASS_KERNEL_TRACE")))
    last_results = res

    out = np.empty((B, S, D), np.float32)
    for b in range(B):
        out[b] = (res.results[2 * b]["out"].astype(np.float32)
                  + res.results[2 * b + 1]["out"].astype(np.float32))
    return out


# revision 56
# speedup vs baseline: 1.0095x; 1.0002x over previous
"""Multi-head attention (B=4, S=2048, D=1024, H=16, d_k=64) on 8 TRN2 NeuronCores.

Sharding: batch x head-group. Core c handles batch b = c//2 and heads
[8*(c%2), 8*(c%2)+8). Each core computes Q/K/V projections for its 512
output features (column-parallel), attention for its 8 heads, and a
row-parallel partial of the W_o output projection. The host sums the two
partials per batch (the row-parallel unshard) — no collectives needed.

Device schedule (per core): one software-pipelined stream. Attention is
split into 256 groups (4 q-chunks x 4 head-pairs x 16 key-blocks). Per
group the PE runs a row-tiled score pair (both heads share one N=512
moving pass) and 2 attn@V matmuls; the ACT engine runs one 1024-free
exp per group (~274us total). Scores for TWO consecutive groups are
emitted back-to-back so the ~120ns post-row-tiled-pair drain stall is
paid once per 2 groups. All other PE work (Q/K/V projections, W_o,
normalization broadcasts) is a deadline-sorted task queue pumped ~2
matmuls per group through a 1-bank misc slot, keeping the PE gapless
(p-state: any PE gap drops the clock 2.4->1.2GHz for ~3us, so dummy
matmuls on a memset tile hold the clock hot through the DMA-bound
prologue and the final reciprocal chain). All DMAs ride the single SP
trigger ring in strict first-consumption order: the ring drains FIFO
at the aggregate ~360GB/s, so order IS the bandwidth schedule for the
11.5MB the first 16 attention groups touch; late tensors (xq1-3, wo)
sit at the ring tail. Output partials are written bf16 (host sums the
two head-group partials in fp32), halving the tail flush. Softmax
denominators ride as a 65th V
row; per pair they are DMA-repacked [1,1024]->[8,128] so the DVE
reciprocal_approx_fast (18-bit, plenty for bf16 downstream) takes
~0.3us, then broadcast across partitions by four [8,128]-stationary
mask matmuls; normalization and W_o are deferred (~1.5 pairs / 1
q-chunk) so their inputs are always long ready. PSUM: 2 rotating
2-bank score slots + 2 attn@V accumulator banks + 2 misc banks.
"""

import os
from collections import defaultdict

import numpy as np
import ml_dtypes

import concourse.bacc as bacc
import concourse.mybir as mybir
import concourse.tile as tile
from concourse.bass_utils import run_bass_kernel_spmd

BF16 = mybir.dt.bfloat16
F32 = mybir.dt.float32
EXP = mybir.ActivationFunctionType.Exp

B, S, D = 4, 2048, 1024
H, DK = 16, 64
HPC = 8           # heads per core
FPC = HPC * DK    # 512 features per core
NP = 4            # head pairs per core
NB = 8            # din blocks of 128
NKB = 16          # key blocks of 128
QC = 512          # query chunk
NQC = S // QC     # 4
NG = NKB          # groups (1 key block each) per (qc, m)

_nc_cache = None
last_results = None


def build():
    nc = bacc.Bacc("TRN2", target_bir_lowering=False, debug=False, num_devices=8)

    xq = nc.dram_tensor("xq", [D, S], BF16, kind="ExternalInput").ap()
    xk = nc.dram_tensor("xk", [D, S], BF16, kind="ExternalInput").ap()
    xv = nc.dram_tensor("xv", [D, S], BF16, kind="ExternalInput").ap()
    wq = nc.dram_tensor("wq", [D, FPC], BF16, kind="ExternalInput").ap()
    wk = nc.dram_tensor("wk", [D, FPC], BF16, kind="ExternalInput").ap()
    wv = nc.dram_tensor("wv", [D, FPC], BF16, kind="ExternalInput").ap()
    wo = nc.dram_tensor("wo", [FPC, D], BF16, kind="ExternalInput").ap()
    mask = nc.dram_tensor("mask", [8, 512], BF16, kind="ExternalInput").ap()
    # bf16 partials: host sums the two head-group partials in fp32; the
    # ~0.2% partial quantization is far inside the error budget and halves
    # the output DMA flush at the kernel tail.
    out = nc.dram_tensor("out", [S, D], BF16, kind="ExternalOutput").ap()

    SP = nc.sync

    with tile.TileContext(nc) as tc:
        with (
            tc.tile_pool(name="wp", bufs=1) as wp,
            tc.tile_pool(name="qkv", bufs=1) as qkv,
            tc.tile_pool(name="xp", bufs=1) as xp,
            tc.tile_pool(name="xvp", bufs=4) as xvp,
            tc.tile_pool(name="ptp", bufs=4) as ptp,
            tc.tile_pool(name="avsb", bufs=2) as avsb,
            tc.tile_pool(name="otp", bufs=2) as otp,
            tc.tile_pool(name="denp", bufs=2) as denp,
            tc.tile_pool(name="recp", bufs=2) as recp,
            tc.tile_pool(name="outp", bufs=3) as outp,
            tc.tile_pool(name="sp", bufs=2, space="PSUM") as sp,
            tc.tile_pool(name="avp", bufs=2, space="PSUM") as avp,
            tc.tile_pool(name="miscp", bufs=2, space="PSUM") as miscp,
        ):
            # ---- static SBUF tensors ----
            wq_sb = wp.tile([128, NB, NP, 128], BF16, tag="wq")
            wk_sb = wp.tile([128, NB, NP, 128], BF16, tag="wk")
            wv_sb = wp.tile([128, NB, FPC], BF16, tag="wv")
            wo_sb = wp.tile([128, NP, D], BF16, tag="wo")
            m_sb = wp.tile([8, NP, 128], BF16, tag="mask")
            dum_sb = wp.tile([128, 512], BF16, tag="dummy")
            qt_sb = qkv.tile([128, NP, S], BF16, tag="qt")
            kt_sb = qkv.tile([128, NP, S], BF16, tag="kt")
            v_sb = qkv.tile([128, NKB, HPC, 65], BF16, tag="v")
            xq_sb = xp.tile([128, NB, S], BF16, tag="xq")
            xk_sb = xp.tile([128, NB, S], BF16, tag="xk")

            xv_ch = {}
            for c in range(4):
                xv_ch[c] = xvp.tile([128, NB, 512], BF16, tag="xv", name=f"xv{c}")

            # dummy tile + ACT exp-table priming (both engine-local, no DMA)
            nc.vector.memset(dum_sb[:], 0.01)
            nc.vector.memset(v_sb[:, :, :, 64], 1.0)
            warm_act = denp.tile([8, 128], F32, tag="den", name="warmact")
            nc.scalar.activation(warm_act[:, 0:64], dum_sb[0:8, 0:64], EXP,
                                 scale=0.125)

            def dummy_mm(n=512):
                ps = miscp.tile([128, 512], F32, tag="misc", name="warm")
                nc.tensor.matmul(ps[:, 0:n], dum_sb[:, 0:128], dum_sb[:, 0:n],
                                 start=True, stop=True)

            # ---- input DMA units, spread across 3 trigger queues ----
            # Each unit is one dma_start; per-queue emission order is
            # first-consumption order. Units sized so each lands fast on its
            # ~22GB/s DMA queue. (SP ~565ns, ACT ~667ns, GPSIMD ~1us per
            # trigger, serially per queue, running in parallel.)
            def u_w(e, w_sb_, w_, m, b0, nb):
                e.dma_start(
                    w_sb_[:, b0:b0 + nb, m],
                    w_[:, m * 128:(m + 1) * 128]
                    .rearrange("(b p) c -> p b c", p=128)[:, b0:b0 + nb])

            def u_x(e, x_sb_, x_, c, b0, nb, t0=0, nt=512):
                x_sb_dst = x_sb_[:, b0:b0 + nb, c * 512 + t0:c * 512 + t0 + nt]
                e.dma_start(
                    x_sb_dst,
                    x_[:, c * 512 + t0:c * 512 + t0 + nt]
                    .rearrange("(b p) t -> p b t", p=128)[:, b0:b0 + nb])

            def u_wv(e, b0, nb):
                e.dma_start(wv_sb[:, b0:b0 + nb],
                            wv.rearrange("(b p) c -> p b c", p=128)[:, b0:b0 + nb])

            def u_xv(e, c, t0, nt):
                e.dma_start(
                    xv_ch[c][:, :, t0:t0 + nt],
                    xv[:, c * 512 + t0:c * 512 + t0 + nt]
                    .rearrange("(b p) t -> p b t", p=128))

            def u_wo(e, f0, nf):
                e.dma_start(wo_sb[:, f0:f0 + nf],
                            wo.rearrange("(f p) c -> p f c", p=128)[:, f0:f0 + nf])

            # DMA plan (single SP ring, baseline-proven): the ring drains
            # dma_starts in FIFO order at aggregate ~360GB/s, and the ring
            # has bounded capacity — the sequencer backpressures when it is
            # full, so trigger issue self-paces to drain rate. A single
            # ring in exact first-consumption order is therefore optimal
            # for the bandwidth-bound first ~30us (the first 16 attention
            # groups touch all of K and V: ~11.5MB). Splitting across the
            # ACT ring halves SP's drain share and delays the stream;
            # GPSIMD (software DGE) DMA is NOT dependency-tracked by the
            # tile framework — never use either for loads.
            SP.dma_start(m_sb[:], mask.rearrange("p (j c) -> p j c", c=128))
            u_w(SP, wq_sb, wq, 0, 0, 4)
            u_w(SP, wq_sb, wq, 0, 4, 4)
            for b in range(4):
                u_x(SP, xq_sb, xq, 0, b, 1)
            u_w(SP, wk_sb, wk, 0, 0, 4)
            u_w(SP, wk_sb, wk, 0, 4, 4)
            for b in range(4, 8):
                u_x(SP, xq_sb, xq, 0, b, 1)
            for b in range(6):
                u_x(SP, xk_sb, xk, 0, b, 1)
            u_wv(SP, 0, 4)
            u_wv(SP, 4, 4)
            u_x(SP, xk_sb, xk, 0, 6, 2)
            for t in range(4):
                u_xv(SP, 0, 128 * t, 128)
            u_x(SP, xk_sb, xk, 1, 0, 4)
            u_x(SP, xk_sb, xk, 1, 4, 4)
            for t in range(4):
                u_xv(SP, 1, 128 * t, 128)
            u_x(SP, xk_sb, xk, 2, 0, 4)
            u_x(SP, xk_sb, xk, 2, 4, 4)
            u_x(SP, xk_sb, xk, 3, 0, 4)
            u_x(SP, xk_sb, xk, 3, 4, 4)
            u_w(SP, wq_sb, wq, 1, 0, NB)
            u_w(SP, wk_sb, wk, 1, 0, NB)
            for t in range(4):
                u_xv(SP, 2, 128 * t, 128)
            for t in range(4):
                u_xv(SP, 3, 128 * t, 128)
            u_w(SP, wq_sb, wq, 2, 0, NB)
            u_w(SP, wk_sb, wk, 2, 0, NB)
            u_w(SP, wq_sb, wq, 3, 0, NB)
            u_w(SP, wk_sb, wk, 3, 0, NB)
            u_x(SP, xq_sb, xq, 1, 0, 4)
            u_x(SP, xq_sb, xq, 1, 4, 4)
            u_wo(SP, 0, 2)
            u_wo(SP, 2, 2)
            u_x(SP, xq_sb, xq, 2, 0, 4)
            u_x(SP, xq_sb, xq, 2, 4, 4)
            u_x(SP, xq_sb, xq, 3, 0, 4)
            u_x(SP, xq_sb, xq, 3, 4, 4)

            # ---- filler tasks: generators yielding one PE matmul per step ----
            def proj_chunk_task(x_sb, w_sb, dst, m, c, act_evac=False):
                """512-token projection chunk: 8 matmuls + evac copy.

                Early tasks (first-q-chunk window, where the PE is the
                bottleneck and ACT has ~20% slack) evacuate via the ACT
                engine: a congested DVE queue otherwise delays the misc
                PSUM slot hand-off and inflates the next task's first
                matmul by ~120ns."""
                ps = miscp.tile([128, 512], F32, tag="misc", name="projps")
                lo = c * 512
                for b in range(NB):
                    nc.tensor.matmul(ps[:], w_sb[:, b, m], x_sb[:, b, lo:lo + 512],
                                     start=(b == 0), stop=(b == NB - 1))
                    if b < NB - 1:
                        yield
                if act_evac:
                    nc.scalar.copy(dst[:, m, lo:lo + 512], ps[:])
                else:
                    nc.vector.tensor_copy(dst[:, m, lo:lo + 512], ps[:])

            def v_chunk_task(tt):
                """V projection for token tile tt: 8 matmuls + evac copy."""
                ps = miscp.tile([128, 512], F32, tag="misc", name="vps")
                ch = xv_ch[tt // 4]
                off = (tt % 4) * 128
                for b in range(NB):
                    nc.tensor.matmul(ps[:], ch[:, b, off:off + 128], wv_sb[:, b],
                                     start=(b == 0), stop=(b == NB - 1))
                    if b < NB - 1:
                        yield
                nc.vector.tensor_copy(
                    v_sb[:, tt, :, 0:64],
                    ps[:].rearrange("p (h c) -> p h c", c=64))

            ot_tiles = {}
            avsb_tiles = {}
            rec_tiles = {}

            def norm_task(qc, m):
                """Normalize pair (qc, m): broadcast 1/den via mask matmul, mul."""
                rec2 = rec_tiles[(qc, m)]
                if m == 0:
                    ot_tiles[qc] = otp.tile([128, NP, QC], BF16, tag="ot", name="ot")
                ot = ot_tiles[qc]
                av_sb = avsb_tiles.pop((qc, m))
                scp = miscp.tile([128, 512], F32, tag="misc", name="scp")
                for j in range(4):
                    nc.tensor.matmul(scp[:, 128 * j:128 * (j + 1)], m_sb[:, j],
                                     rec2[:], start=True, stop=True)
                nc.vector.tensor_mul(ot[0:64, m], av_sb[0:64, 0:QC], scp[0:64, 0:QC])
                nc.vector.tensor_mul(ot[64:128, m], av_sb[0:64, QC:2 * QC], scp[64:128, 0:QC])
                return
                yield

            def wo_task(qc, tt, jc):
                """Half of the output projection for token block (qc, tt)."""
                ot = ot_tiles[qc]
                wop = miscp.tile([128, 512], F32, tag="misc", name="wop")
                tsl = slice(tt * 128, (tt + 1) * 128)
                for fb in range(NP):
                    nc.tensor.matmul(
                        wop[:], ot[:, fb, tsl], wo_sb[:, fb, jc * 512:(jc + 1) * 512],
                        start=(fb == 0), stop=(fb == NP - 1))
                    if fb < NP - 1:
                        yield
                # per-(tt,jc) staging halves: outp bufs=3 rotation means a
                # copy only waits for the DMA three halves back. At the
                # drain the evacs alternate ACT/DVE so neither serializes.
                ostage = outp.tile([128, 512], BF16, tag="ostage", name="ostage")
                if qc == NQC - 1 and (tt + jc) % 2:
                    nc.scalar.copy(ostage[:], wop[:])
                else:
                    nc.vector.tensor_copy(ostage[:], wop[:])
                row = qc * QC + tt * 128
                SP.dma_start(out[row:row + 128, jc * 512:(jc + 1) * 512],
                             ostage[:])

            ostage_tiles = {}

            # Task queue: (due_index, generator). Pump keeps deadline order.
            tasks = []
            def add_task(due, gen):
                tasks.append((due, gen))
                tasks.sort(key=lambda t: t[0])

            open_task = None
            open_due = 0

            def pump(i, min_steps):
                """Emit filler matmul-steps: min_steps per index, running
                ahead only to finish overdue tasks (deadline-driven)."""
                nonlocal open_task, open_due
                emitted = 0
                while True:
                    if open_task is None and tasks and (
                            emitted < min_steps or tasks[0][0] <= i + 2):
                        open_due, open_task = tasks.pop(0)
                    if open_task is None:
                        return
                    if emitted >= min_steps and open_due > i:
                        return
                    try:
                        next(open_task)
                    except StopIteration:
                        open_task = None
                    emitted += 1

            # Due = first index whose scores/av READ the produced tile, minus
            # 5: the pump runs after score emission within each 2-group
            # macro, so tasks must finish a full macro earlier than with the
            # old per-group loop to keep consumers off the evac-copy wait.
            for tt in range(NKB):
                if tt >= 1:
                    add_task(1 + tt, v_chunk_task(tt))
            for m in range(NP):
                for c in range(4):
                    if not (m == 0 and c == 0):
                        add_task(16 * m + 4 * c - 5,
                                 proj_chunk_task(xk_sb, wk_sb, kt_sb, m, c))
            for m in range(NP):
                for c in range(4):
                    if not (m == 0 and c == 0):
                        add_task(64 * c + 16 * m - 5,
                                 proj_chunk_task(xq_sb, wq_sb, qt_sb, m, c))

            fill_at = defaultdict(list)
            # normalization / Wo release points: pair p=(qc, m) spans indices
            # [16p, 16p+16). norm of pair p-2 at +10 (reciprocal chain of p-2
            # finished ~5 indices earlier); Wo of q-chunk qc-1 in (qc, m2/m3).
            for qc in range(NQC):
                for m in range(NP):
                    base = 16 * (4 * qc + m)
                    if m >= 2 or qc > 0:
                        nqc, nm = (qc, m - 2) if m >= 2 else (qc - 1, m + 2)
                        if nqc < NQC - 1:
                            fill_at[base + 10].append(
                                lambda nqc=nqc, nm=nm: norm_task(nqc, nm))
                    if qc > 0 and m >= 2:
                        tt0 = 2 * (m - 2)
                        fill_at[base + 2].append(
                            lambda q=qc - 1, tt=tt0: wo_task(q, tt, 0))
                        fill_at[base + 5].append(
                            lambda q=qc - 1, tt=tt0: wo_task(q, tt, 1))
                        fill_at[base + 9].append(
                            lambda q=qc - 1, tt=tt0 + 1: wo_task(q, tt, 0))
                        fill_at[base + 12].append(
                            lambda q=qc - 1, tt=tt0 + 1: wo_task(q, tt, 1))
            # last q-chunk: per-pair reciprocal lets each norm release ~6
            # indices after its pair ends (norm(3,3) + all Wo(3,*) drain)
            for m in range(NP - 1):
                fill_at[16 * (4 * (NQC - 1) + m) + 22].append(
                    lambda nm=m: norm_task(NQC - 1, nm))


            # ---- prologue: warm the PE p-state on dummy matmuls while the
            # first DMA chunks land, then minimum projections for attention.
            # Dummies are interleaved between the DMA-paced first matmuls so
            # the clock never drops (a gap halves the PE rate for ~3us).
            for _ in range(16):
                dummy_mm(128)
            for _ in range(8):
                dummy_mm(512)

            def prologue_chunk(gen, fillers):
                for _ in gen:
                    for _ in range(fillers):
                        dummy_mm(512)

            prologue_chunk(proj_chunk_task(xq_sb, wq_sb, qt_sb, 0, 0), 2)
            for _ in range(3):
                dummy_mm(512)
            prologue_chunk(proj_chunk_task(xk_sb, wk_sb, kt_sb, 0, 0), 1)
            # v(0) is not needed until av(0) in macro 2 — pump it there so
            # scores(0)/exp(0) start as soon as q00/k00 are projected
            add_task(-1, v_chunk_task(0))

            # ---- main attention pipeline ----
            groups = [(qc, m, g) for qc in range(NQC) for m in range(NP)
                      for g in range(NG)]
            NGRP = len(groups)
            SKEW = 3
            pt_tiles = {}
            av_tiles = {}

            def emit_scores(gi):
                qc, m, g = groups[gi]
                qsl = slice(qc * QC, (qc + 1) * QC)
                ksl = slice(g * 128, (g + 1) * 128)
                s = sp.tile([128, 1024], F32, tag="s", name="s")
                nc.tensor.matmul(s[:, 0:512], kt_sb[0:64, m, ksl], qt_sb[0:64, m, qsl],
                                 start=True, stop=True, tile_position=(0, 0))
                nc.tensor.matmul(s[:, 512:1024], kt_sb[64:128, m, ksl], qt_sb[64:128, m, qsl],
                                 start=True, stop=True, tile_position=(64, 0))
                pt = ptp.tile([128, 1024], BF16, tag="pt", name="pt")
                pt_tiles[gi] = pt
                nc.scalar.activation(pt[:], s[:], EXP, scale=0.125)

            def emit_av(gi):
                qc, m, g = groups[gi]
                pt = pt_tiles.pop(gi)
                if g == 0:
                    av_tiles[(qc, m, 0)] = avp.tile([128, QC], F32, tag="av", name="avA")
                    av_tiles[(qc, m, 1)] = avp.tile([128, QC], F32, tag="av", name="avB")
                avA = av_tiles[(qc, m, 0)]
                avB = av_tiles[(qc, m, 1)]
                nc.tensor.matmul(avA[0:65, :], v_sb[:, g, 2 * m, 0:65], pt[:, 0:512],
                                 start=(g == 0), stop=(g == NG - 1))
                nc.tensor.matmul(avB[0:65, :], v_sb[:, g, 2 * m + 1, 0:65], pt[:, 512:1024],
                                 start=(g == 0), stop=(g == NG - 1))
                if g == NG - 1:
                    pair_end(qc, m)

            def pair_end(qc, m):
                last = (qc == NQC - 1 and m == NP - 1)
                avA = av_tiles.pop((qc, m, 0))
                avB = av_tiles.pop((qc, m, 1))
                av_sb = avsb.tile([128, 2 * QC], F32, tag="av_sb", name="av_sb")
                avsb_tiles[(qc, m)] = av_sb
                den = denp.tile([8, 128], F32, tag="den", name="den")
                # denominators: [1, 512] -> [4, 128] repack per head keeps the
                # reciprocal's free size small; head A lands while head B's
                # copy still runs on the DVE.
                nc.vector.tensor_copy(av_sb[0:65, 0:QC], avA[0:65, :])
                SP.dma_start(den[0:4], av_sb[64:65, 0:QC])
                nc.vector.tensor_copy(av_sb[0:65, QC:2 * QC], avB[0:65, :])
                SP.dma_start(den[4:8], av_sb[64:65, QC:2 * QC])
                recf = recp.tile([8, 128], F32, tag="recf", name="recf")
                nc.vector.reciprocal_approx_fast(recf[:], den[:])
                rec2 = recp.tile([8, 128], BF16, tag="rec2", name="rec2")
                nc.vector.tensor_copy(rec2[:], recf[:])
                rec_tiles[(qc, m)] = rec2

            av_cursor = 0
            for j in range(0, NGRP + 4, 2):
                for i in (j, j + 1):
                    for f in fill_at.pop(i, []):
                        add_task(i, f())
                if j < NGRP:
                    emit_scores(j)
                    emit_scores(j + 1)
                if j < 24:
                    if j < 12:
                        dummy_mm(512)   # p-state insurance in early DMA waits
                    pump(j + 1, 6)
                    while av_cursor <= min(j + 1 - SKEW, NGRP - 1):
                        emit_av(av_cursor)
                        av_cursor += 1
                else:
                    quota = 4 if j < NGRP else NGRP
                    while quota and av_cursor <= min(j + 1 - SKEW, NGRP - 1):
                        emit_av(av_cursor)
                        av_cursor += 1
                        quota -= 1
                    pump(j + 1, 4)

            # ---- drain: remaining tasks, dummies to keep the PE clock hot
            # through the last pair's reciprocal chain, then norm + Wo ----
            pump(10 ** 9, 10 ** 9)
            # Wo(3): pairs 0-2 of six blocks accumulate into the freed
            # score/av PSUM banks while the last pair's reciprocal chain
            # runs; after norm(3,3) each such block needs only pair 3's
            # matmul + evac. Two blocks stay as plain misc-slot tasks (the
            # norm's scp broadcast needs a misc bank for itself).
            s0 = sp.tile([128, 1024], F32, tag="s", name="wod0")
            s1 = sp.tile([128, 1024], F32, tag="s", name="wod1")
            a0 = avp.tile([128, QC], F32, tag="av", name="wod2")
            a1 = avp.tile([128, QC], F32, tag="av", name="wod3")
            pviews = [s0[:, 0:512], s0[:, 512:1024],
                      s1[:, 0:512], s1[:, 512:1024], a0[:, :], a1[:, :]]
            pblocks = [(tt, jc) for tt in range(3) for jc in range(2)]
            ot3 = ot_tiles[NQC - 1]
            for (tt, jc), pv in zip(pblocks, pviews):
                tsl = slice(tt * 128, (tt + 1) * 128)
                for fb in range(3):
                    nc.tensor.matmul(pv, ot3[:, fb, tsl],
                                     wo_sb[:, fb, jc * 512:(jc + 1) * 512],
                                     start=(fb == 0), stop=False)
            for _ in norm_task(NQC - 1, 3):
                pass
            for k, ((tt, jc), pv) in enumerate(zip(pblocks, pviews)):
                tsl = slice(tt * 128, (tt + 1) * 128)
                nc.tensor.matmul(pv, ot3[:, 3, tsl],
                                 wo_sb[:, 3, jc * 512:(jc + 1) * 512],
                                 start=False, stop=True)
                ostage = outp.tile([128, 512], BF16, tag="ostage", name="ostage")
                if k % 2:
                    nc.scalar.copy(ostage[:], pv)
                else:
                    nc.vector.tensor_copy(ostage[:], pv)
                row = (NQC - 1) * QC + tt * 128
                SP.dma_start(out[row:row + 128, jc * 512:(jc + 1) * 512],
                             ostage[:])
            for gen in [wo_task(NQC - 1, 3, jc) for jc in range(2)]:
                for _ in gen:
                    pass

    nc.compile()
    return nc


def _get_nc():
    global _nc_cache
    if _nc_cache is None:
        _nc_cache = build()
    return _nc_cache


def kernel(query, key, value, W_q, W_k, W_v, W_o):
    global last_results
    nc = _get_nc()
    bf = ml_dtypes.bfloat16

    # broadcast masks for the [8, 128]-packed reciprocals: output column
    # block j picks row j (head A, partitions 0:64) / row 4+j (head B)
    mask = np.zeros((8, 512), bf)
    for j in range(4):
        mask[j, j * 128:j * 128 + 64] = 1.0
        mask[4 + j, j * 128 + 64:j * 128 + 128] = 1.0

    in_maps = []
    xt = {}
    for b in range(B):
        xt[b] = {
            "xq": np.ascontiguousarray(query[b].T).astype(bf),
            "xk": np.ascontiguousarray(key[b].T).astype(bf),
            "xv": np.ascontiguousarray(value[b].T).astype(bf),
        }
    wmaps = []
    for hg in range(2):
        r = slice(hg * FPC, (hg + 1) * FPC)
        wmaps.append({
            "wq": np.ascontiguousarray(W_q[r, :].T).astype(bf),
            "wk": np.ascontiguousarray(W_k[r, :].T).astype(bf),
            "wv": np.ascontiguousarray(W_v[r, :].T).astype(bf),
            "wo": np.ascontiguousarray(W_o[:, r].T).astype(bf),
        })
    for c in range(8):
        b, hg = c // 2, c % 2
        in_maps.append({**xt[b], **wmaps[hg], "mask": mask})

    res = run_bass_kernel_spmd(
        nc, in_maps, core_ids=list(range(8)),
        trace=bool(os.environ.get("BASS_KERNEL_TRACE")))
    last_results = res

    out = np.empty((B, S, D), np.float32)
    for b in range(B):
        out[b] = (res.results[2 * b]["out"].astype(np.float32)
                  + res.results[2 * b + 1]["out"].astype(np.float32))
    return out
